# revision 1
# baseline (speedup 1.0000x reference)
"""Bass/Trainium2 SPMD kernel for DemopackDecoder (vq_codebook).

Math: decoded[t] = mean_k codewords[indices[t,:,k]]        [512, 4096]
      W[t]       = (decoded[t] @ rotations[t]) * scales[t] [512, 4096]
      out        = x @ concat_t(W[t]).T + bias             [512, 4096]

Sharding (8 cores, expert-parallel over tiles t): core t computes the
output column block [512 tok, 512 feat] for its tile; host concatenates.

Device dataflow (v2 — reassociated so decode overlaps the big GEMM):
  out_t = x @ rot_t^T @ dec_t^T          (rot pre-scaled by s_t/4 on host)
  G1: zT[e,n] = rot_t[d,e]^T-blocks @ xT[d,n]   -- depends only on x+rot,
      so it starts immediately; rot streams from HBM in fp16.
  A : indirect-DMA gather codeword rows (deduped local codebook, fp16) +
      DVE adds -> dec [r,d]; PE-transpose 128x128 tiles -> decT resident.
      All of A hides under G1 (DMA/DVE/ACT run beside the PE).
  G2: out[r,n] += decT[e,r-block]^T @ zT[e,n]    (128 MMs, tail)
  C : + bias, DMA out.
All matmul operands fp16 (1 PE cycle/row, half the HBM traffic of fp32;
rel err ~1e-3 vs the 2e-2 gate). PSUM accumulation is fp32 throughout.
"""

import contextlib

import numpy as np

import concourse.bass as bass
import concourse.mybir as mybir
import concourse.tile as tile
from concourse import bacc, bass_utils
from concourse.masks import make_identity

# Problem shapes (hardcoded per contract)
T, R, K, D = 8, 512, 4, 4096
N_CW, N_TOK, N_CORES = 16384, 512, 8
P = 128
LCW_ROWS = 2048          # padded dedup codebook rows per core
KT = D // P              # 32 contraction (d) tiles
MT = D // P              # 32 rotated-feature (e) tiles
RT = R // P              # 4 decoded row tiles
JT = R // P              # 4 local out-feature tiles
XQ = 8                   # x loaded in 8 chunks of 4 k-blocks

_PROGRAM_CACHE = {}

# transpose pacing: 8 per m-block starting here (144 slots for 128 tps)
TP_START = 14
TP_PER_BLK = 8


def _build_program(loop_n=1):
    f32 = mybir.dt.float32
    f16 = mybir.dt.float16
    i32 = mybir.dt.int32

    nc = bacc.Bacc("TRN2", target_bir_lowering=False, debug=False)
    lcw = nc.dram_tensor("lcw", [LCW_ROWS, D], f16, kind="ExternalInput").ap()
    ridx = nc.dram_tensor("ridx", [P, RT * K], i32, kind="ExternalInput").ap()
    rtb = nc.dram_tensor("rtb", [MT * P, D], f16, kind="ExternalInput").ap()
    xT = nc.dram_tensor("xT", [D, N_TOK], f16, kind="ExternalInput").ap()
    biasd = nc.dram_tensor("biasd", [P, JT], f32, kind="ExternalInput").ap()
    outT = nc.dram_tensor("outT", [R, N_TOK], f32, kind="ExternalOutput").ap()

    # DRAM views
    rtb_v = rtb.rearrange("(m p) d -> m p d", p=P)         # [32, 128, 4096]
    xT_v = xT.rearrange("(q j p) n -> q p j n", j=4, p=P)  # [8, 128, 4, 512]
    outT_v = outT.rearrange("(j p) n -> p j n", p=P)       # [128, 4, 512]

    with tile.TileContext(nc) as tc:
        with (
            tc.tile_pool(name="const", bufs=1) as cpool,
            tc.tile_pool(name="xbuf", bufs=XQ) as xpool,
            tc.tile_pool(name="zbuf", bufs=MT) as zpool,
            tc.tile_pool(name="decT", bufs=KT) as dpool,
            tc.tile_pool(name="rbuf", bufs=4) as rpool,
            tc.tile_pool(name="accp", bufs=RT) as apool,
            tc.tile_pool(name="gath", bufs=2) as gpool,
            tc.tile_pool(name="outp", bufs=1) as opool,
            tc.tile_pool(name="psZ", bufs=2, space="PSUM") as psZ,
            tc.tile_pool(name="psA", bufs=2, space="PSUM") as psA,
            tc.tile_pool(name="psO", bufs=4, space="PSUM") as psO,
        ):
            ident = cpool.tile([P, P], f16, tag="ident")
            make_identity(nc, ident[:])
            ridx_sb = cpool.tile([P, RT * K], i32, tag="ridx_sb")
            nc.sync.dma_start(ridx_sb[:], ridx)
            bias_sb = cpool.tile([P, JT], f32, tag="bias_sb")
            nc.sync.dma_start(bias_sb[:], biasd)

            loop_cm = tc.For_i(0, loop_n, 1) if loop_n > 1 else contextlib.nullcontext()
            with loop_cm:
                _emit_body(nc, tc, locals())

    nc.compile()
    return nc


def _emit_body(nc, tc, env, phases="full", g1dt=None):
    f32 = mybir.dt.float32
    f16 = mybir.dt.float16
    if g1dt is None:
        g1dt = f16
    lcw, ridx_sb, bias_sb, ident = env["lcw"], env["ridx_sb"], env["bias_sb"], env["ident"]
    rtb_v, xT_v, outT_v = env["rtb_v"], env["xT_v"], env["outT_v"]
    xpool, zpool, dpool, rpool = env["xpool"], env["zpool"], env["dpool"], env["rpool"]
    apool, gpool, opool = env["apool"], env["gpool"], env["opool"]
    psZ, psA, psO = env["psZ"], env["psA"], env["psO"]

    # resident SBUF tiles
    xsb = [xpool.tile([P, 4 * N_TOK], g1dt, tag="xsb", name=f"xsb{q}") for q in range(XQ)]
    zT = [zpool.tile([P, N_TOK], f16, tag="zT", name=f"zT{m}") for m in range(MT)]
    decT = [dpool.tile([P, R], f16, tag="decT", name=f"decT{k}") for k in range(KT)]

    # x chunks: the m=0 matmuls wait per-chunk, so PE starts after chunk 0
    for q in range(XQ):
        nc.sync.dma_start(
            xsb[q][:].rearrange("p (j n) -> p j n", n=N_TOK), xT_v[q]
        )

    # gather DMAs up front on the gpsimd queue (paced by gpool slots);
    # the DVE adds are spread into the m-loop below
    accs = []
    pending_adds = []
    dec_on = "g1" not in phases
    for i in range(RT if dec_on else 0):
        acc = apool.tile([P, D], f16, tag="acc", name=f"acc{i}")
        nc.gpsimd.indirect_dma_start(
            out=acc[:], out_offset=None, in_=lcw,
            in_offset=bass.IndirectOffsetOnAxis(
                ap=ridx_sb[:, i * K: i * K + 1], axis=0),
        )
        accs.append(acc)
        for k in range(1, K):
            g = gpool.tile([P, D], f16, tag="g")
            nc.gpsimd.indirect_dma_start(
                out=g[:], out_offset=None, in_=lcw,
                in_offset=bass.IndirectOffsetOnAxis(
                    ap=ridx_sb[:, i * K + k: i * K + k + 1], axis=0),
            )
            pending_adds.append((acc, g))

    # transposes: (i, kk) pairs, i-major so group i is fully consumed first
    pending_tps = [(i, kk) for i in range(RT) for kk in range(KT)] if dec_on else []

    # ---- G1: zT[m] = sum_k rtb[m,:,k-block]^T @ xT[k-block,:] ----
    for m in range(MT):
        rm = rpool.tile([P, D], g1dt, tag="rm")
        nc.sync.dma_start(rm[:], rtb_v[m])
        ps = psZ.tile([P, N_TOK], f32, tag="psZ")
        for k in range(KT):
            nc.tensor.matmul(
                ps[:],
                lhsT=rm[:, k * P:(k + 1) * P],
                rhs=xsb[k // 4][:, (k % 4) * N_TOK:(k % 4 + 1) * N_TOK],
                start=(k == 0),
                stop=(k == KT - 1),
            )
        nc.vector.tensor_copy(zT[m][:], ps[:])

        if m >= 1 and pending_adds:
            acc, g = pending_adds.pop(0)
            nc.vector.tensor_add(acc[:], acc[:], g[:])

        if m >= TP_START:
            for _ in range(min(TP_PER_BLK, len(pending_tps))):
                i, kk = pending_tps.pop(0)
                pst = psA.tile([P, P], f16, tag="psA")
                nc.tensor.transpose(
                    out=pst[:],
                    in_=accs[i][:, kk * P:(kk + 1) * P],
                    identity=ident[:],
                )
                nc.scalar.copy(decT[kk][:, i * P:(i + 1) * P], pst[:])

    # safety net (should be empty with TP_START=14, TP_PER_BLK=8)
    for acc, g in pending_adds:
        nc.vector.tensor_add(acc[:], acc[:], g[:])
    for i, kk in pending_tps:
        pst = psA.tile([P, P], f16, tag="psA")
        nc.tensor.transpose(out=pst[:], in_=accs[i][:, kk * P:(kk + 1) * P],
                            identity=ident[:])
        nc.scalar.copy(decT[kk][:, i * P:(i + 1) * P], pst[:])

    if not dec_on:
        # G1-only timing probe: consume zT so it can't be DCE'd
        out_sb = opool.tile([P, JT * N_TOK], f32, tag="osb")
        for m in range(MT):
            nc.vector.tensor_copy(
                out_sb[:, (m % JT) * N_TOK:(m % JT + 1) * N_TOK], zT[m][:])
        nc.sync.dma_start(
            outT_v, out_sb[:].rearrange("p (j n) -> p j n", n=N_TOK))
        return

    # ---- G2: out[r,n] = sum_e decT[e,r]^T @ zT[e,n] ----
    out_ps = [psO.tile([P, N_TOK], f32, tag="psO", name=f"outps{j}")
              for j in range(JT)]
    for m in range(MT):
        for j in range(JT):
            nc.tensor.matmul(
                out_ps[j][:],
                lhsT=decT[m][:, j * P:(j + 1) * P],
                rhs=zT[m][:],
                start=(m == 0),
                stop=(m == MT - 1),
            )

    # ---- C: bias + store, per-j so the DMA overlaps the next bias add ----
    out_sb = opool.tile([P, JT * N_TOK], f32, tag="osb")
    for j in range(JT):
        nc.vector.tensor_scalar(
            out=out_sb[:, j * N_TOK:(j + 1) * N_TOK],
            in0=out_ps[j][:],
            scalar1=bias_sb[:, j:j + 1],
            scalar2=None,
            op0=mybir.AluOpType.add,
        )
        nc.sync.dma_start(
            outT_v[:, j], out_sb[:, j * N_TOK:(j + 1) * N_TOK]
        )


def _get_program(loop_n=1):
    if loop_n not in _PROGRAM_CACHE:
        _PROGRAM_CACHE[loop_n] = _build_program(loop_n)
    return _PROGRAM_CACHE[loop_n]


def _build_bench_program(loop_n, phases="full", g1dt_name="float16"):
    """Timing-only variant: big tensors are Internal (device-resident, no
    per-call upload over axon — kills the transfer noise that swamps the
    loop delta) and the body repeats a static loop_n times. Values in
    lcw/rtb/xT are garbage; engine timing is data-independent. ridx stays
    a real external input (it feeds DMA offsets, which must stay
    in-bounds)."""
    f32 = mybir.dt.float32
    f16 = mybir.dt.float16
    i32 = mybir.dt.int32

    g1dt = getattr(mybir.dt, g1dt_name)
    nc = bacc.Bacc("TRN2", target_bir_lowering=False, debug=False)
    lcw = nc.dram_tensor("lcw", [LCW_ROWS, D], f16, kind="Internal").ap()
    ridx = nc.dram_tensor("ridx", [P, RT * K], i32, kind="ExternalInput").ap()
    rtb = nc.dram_tensor("rtb", [MT * P, D], g1dt, kind="Internal").ap()
    xT = nc.dram_tensor("xT", [D, N_TOK], g1dt, kind="Internal").ap()
    biasd = nc.dram_tensor("biasd", [P, JT], f32, kind="ExternalInput").ap()
    outT = nc.dram_tensor("outT", [R, N_TOK], f32, kind="ExternalOutput").ap()

    rtb_v = rtb.rearrange("(m p) d -> m p d", p=P)
    xT_v = xT.rearrange("(q j p) n -> q p j n", j=4, p=P)
    outT_v = outT.rearrange("(j p) n -> p j n", p=P)

    with tile.TileContext(nc) as tc:
        with (
            tc.tile_pool(name="const", bufs=1) as cpool,
            tc.tile_pool(name="xbuf", bufs=XQ) as xpool,
            tc.tile_pool(name="zbuf", bufs=MT) as zpool,
            tc.tile_pool(name="decT", bufs=KT) as dpool,
            tc.tile_pool(name="rbuf", bufs=4) as rpool,
            tc.tile_pool(name="accp", bufs=RT) as apool,
            tc.tile_pool(name="gath", bufs=2) as gpool,
            tc.tile_pool(name="outp", bufs=1) as opool,
            tc.tile_pool(name="psZ", bufs=2, space="PSUM") as psZ,
            tc.tile_pool(name="psA", bufs=2, space="PSUM") as psA,
            tc.tile_pool(name="psO", bufs=4, space="PSUM") as psO,
        ):
            ident = cpool.tile([P, P], f16, tag="ident")
            make_identity(nc, ident[:])
            ridx_sb = cpool.tile([P, RT * K], i32, tag="ridx_sb")
            nc.sync.dma_start(ridx_sb[:], ridx)
            bias_sb = cpool.tile([P, JT], f32, tag="bias_sb")
            nc.sync.dma_start(bias_sb[:], biasd)

            loop_cm = tc.For_i(0, loop_n, 1) if loop_n > 1 else contextlib.nullcontext()
            with loop_cm:
                _emit_body(nc, tc, locals(), phases=phases, g1dt=g1dt)

    nc.compile()
    return nc


def _get_bench_program(loop_n, phases="full", g1dt_name="float16"):
    key = ("bench", loop_n, phases, g1dt_name)
    if key not in _PROGRAM_CACHE:
        _PROGRAM_CACHE[key] = _build_bench_program(loop_n, phases, g1dt_name)
    return _PROGRAM_CACHE[key]


def _make_in_maps(x, codewords, indices, rotations, scales, bias):
    x = np.asarray(x, dtype=np.float32)
    codewords = np.asarray(codewords, dtype=np.float32)
    indices = np.asarray(indices)
    rotations = np.asarray(rotations, dtype=np.float32)
    scales = np.asarray(scales, dtype=np.float32)
    bias = np.asarray(bias, dtype=np.float32)

    xTh = np.ascontiguousarray(x.T.astype(np.float16))  # [4096, 512]
    in_maps = []
    for t in range(T):
        idx_t = indices[t].reshape(-1).astype(np.int64)
        uniq, inv = np.unique(idx_t, return_inverse=True)
        assert len(uniq) <= LCW_ROWS
        lcw = np.zeros((LCW_ROWS, D), np.float16)
        lcw[: len(uniq)] = codewords[uniq].astype(np.float16)
        inv = inv.reshape(R, K).astype(np.int32)
        ridx = np.zeros((P, RT * K), np.int32)
        for i in range(RT):
            for k in range(K):
                ridx[:, i * K + k] = inv[i * P:(i + 1) * P, k]
        # G1 contracts over rot's SECOND axis (e): z[n,d] = sum_e x[n,e]rot[d,e]
        # lhsT must be rot^T: rtb[m, p, k*128+j] = (rot*s/4)^T[k*128+p, m*128+j]
        rt = (rotations[t] * (scales[t] / K)).T
        rtb = np.ascontiguousarray(
            rt.reshape(KT, P, MT, P).transpose(2, 1, 0, 3)
            .reshape(MT * P, D).astype(np.float16)
        )
        bias_t = np.ascontiguousarray(
            bias[R * t: R * (t + 1)].reshape(JT, P).T
        ).astype(np.float32)
        in_maps.append(
            {"lcw": lcw, "ridx": ridx, "rtb": rtb, "xT": xTh, "biasd": bias_t}
        )
    return in_maps


def kernel(x, codewords, indices, rotations, scales, bias):
    in_maps = _make_in_maps(x, codewords, indices, rotations, scales, bias)
    nc = _get_program()
    res = bass_utils.run_bass_kernel_spmd(nc, in_maps, core_ids=list(range(N_CORES)))
    out = np.empty((N_TOK, T * R), np.float32)
    for t in range(T):
        out[:, R * t: R * (t + 1)] = res.results[t]["outT"].T
    return out


if __name__ == "__main__":
    rng = np.random.default_rng(0)
    ins = {
        "x": rng.standard_normal((N_TOK, D), dtype=np.float32),
        "codewords": rng.standard_normal((N_CW, D), dtype=np.float32) * 0.02,
        "indices": rng.integers(0, N_CW, size=(T, R, K)),
        "rotations": rng.standard_normal((T, D, D), dtype=np.float32) / np.sqrt(D),
        "scales": (rng.random(T, dtype=np.float32) + 0.5),
        "bias": np.zeros(D, np.float32),
    }
    out = kernel(**ins)
    print("out", out.shape, out.dtype, np.abs(out).mean())

